# revision 1
# baseline (speedup 1.0000x reference)
"""Trainium2 Bass kernel for nn_MixtureOfExperts_29867202576447.

Strategy: data-parallel over tokens (8 cores x 512 tokens), dense expert
compute with top-2 gates applied as a mask, float32r (tf32) matmuls,
activations kept in [channels, tokens] layout so every GEMM/conv consumes
the previous layer's output directly (weights stationary as lhsT).

Self-contained: hardcodes all shapes; host-side prep only shards/pads x,
transposes the small conv weights, and packs biases.
"""
import numpy as np
from contextlib import ExitStack

import concourse.bass as bass
import concourse.tile as tile
import concourse.mybir as mybir
from concourse.bass_utils import run_bass_kernel_spmd

F32R = mybir.dt.float32r
F32 = mybir.dt.float32
AF = mybir.ActivationFunctionType
OP = mybir.AluOpType
AX = mybir.AxisListType

D_IN, D_HID, E = 512, 2048, 10
B, T = 2, 2048
TPC = 512          # tokens per core
HALO = 64          # halo columns on each side of the token window
W = TPC + 2 * HALO  # 640 buffer columns per core
NCORES = 8

_ctr = [0]


def _split_multi_waits(nc, max_waits=1):
    """walrus here accepts one sync-wait per instruction; hoist extras onto
    same-engine NoOps placed immediately before the instruction."""
    n = 0
    for f in nc.m.functions:
        for bb in f.blocks:
            out = []
            changed = False
            for ins in bb.instructions:
                si = getattr(ins, "sync_info", None)
                waits = list(si.on_wait) if (si is not None and si.on_wait) else []
                if len(waits) > max_waits:
                    for w in waits[:-max_waits]:
                        _ctr[0] += 1
                        nop = mybir.InstNoOp(
                            name=f"I-waitsplit-{_ctr[0]}", engine=ins.engine,
                            ins=[], outs=[])
                        nop.sync_info = mybir.SyncInfo(on_wait=[w], on_update=[])
                        nc.register_instruction(nop)
                        out.append(nop)
                    si.on_wait = waits[-max_waits:]
                    changed = True
                    n += 1
                out.append(ins)
            if changed:
                bb.instructions = out
    return n


def _build(reps=1):
    nc = bass.Bass(trn_type="TRN2")

    # ---------------- DRAM I/O ----------------
    xt = nc.dram_tensor("xt", [D_IN, W], F32R, kind="ExternalInput")
    rcw1t = nc.dram_tensor("rcw1t", [3, D_IN, D_IN], F32R, kind="ExternalInput")
    rcw2t = nc.dram_tensor("rcw2t", [3, D_IN, E], F32R, kind="ExternalInput")
    shw1t = nc.dram_tensor("shw1t", [9, D_IN, D_HID], F32R, kind="ExternalInput")
    shw2t = nc.dram_tensor("shw2t", [D_HID, D_IN], F32R, kind="ExternalInput")
    ew1 = nc.dram_tensor("ew1", [E, D_IN, D_HID], F32R, kind="ExternalInput")
    ew2 = nc.dram_tensor("ew2", [E, D_HID, D_IN], F32R, kind="ExternalInput")
    rcb1 = nc.dram_tensor("rcb1", [128, 4], F32, kind="ExternalInput")
    rcb2 = nc.dram_tensor("rcb2", [E, 1], F32, kind="ExternalInput")
    shb1 = nc.dram_tensor("shb1", [128, 16], F32, kind="ExternalInput")
    shb2 = nc.dram_tensor("shb2", [128, 4], F32, kind="ExternalInput")
    eb1p = nc.dram_tensor("eb1p", [E, 128, 16], F32, kind="ExternalInput")
    ebcp = nc.dram_tensor("ebcp", [E, 128, 4], F32, kind="ExternalInput")
    lng = nc.dram_tensor("lng", [1, D_IN], F32R, kind="ExternalInput")
    lnb = nc.dram_tensor("lnb", [1, D_IN], F32R, kind="ExternalInput")
    ones = nc.dram_tensor("ones", [1, 128], F32R, kind="ExternalInput")
    sel = nc.dram_tensor("sel", [E, E * 128], F32R, kind="ExternalInput")
    ident = nc.dram_tensor("ident", [128, 128], F32, kind="ExternalInput")
    rcmask = nc.dram_tensor("rcmask", [1, 528], F32R, kind="ExternalInput")
    yout = nc.dram_tensor("yout", [TPC, D_IN], F32, kind="ExternalOutput")

    C0 = HALO            # buffer col of first center token
    with tile.TileContext(nc) as tc:
      for rep in range(reps):
       with ExitStack() as ctx:
        R = f"r{rep}_"
        const = ctx.enter_context(tc.tile_pool(name=R + "const", bufs=1))
        acts = ctx.enter_context(tc.tile_pool(name=R + "acts", bufs=1))
        wstream = ctx.enter_context(tc.tile_pool(name=R + "wstream", bufs=10))
        scratch = ctx.enter_context(tc.tile_pool(name=R + "scratch", bufs=2))
        hpool = ctx.enter_context(tc.tile_pool(name=R + "hpool", bufs=18))
        psum = ctx.enter_context(tc.tile_pool(name=R + "psum", bufs=7, space="PSUM"))
        pst = psum

        # ---------------- constants / x ----------------
        id_sb = const.tile([128, 128], F32)
        nc.sync.dma_start(id_sb[:], ident[:])
        ones_sb = const.tile([1, 128], F32R)
        nc.sync.dma_start(ones_sb[:], ones[:])
        sel_sb = const.tile([E, E * 128], F32R)
        nc.sync.dma_start(sel_sb[:], sel[:])
        rcb1_sb = const.tile([128, 4], F32)
        nc.sync.dma_start(rcb1_sb[:], rcb1[:])
        rcb2_sb = const.tile([E, 1], F32)
        nc.sync.dma_start(rcb2_sb[:], rcb2[:])
        shb1_sb = const.tile([128, 16], F32)
        nc.sync.dma_start(shb1_sb[:], shb1[:])
        shb2_sb = const.tile([128, 4], F32)
        nc.sync.dma_start(shb2_sb[:], shb2[:])
        lng_r = const.tile([1, D_IN], F32R)
        nc.sync.dma_start(lng_r[:], lng[:])
        lnb_r = const.tile([1, D_IN], F32R)
        nc.sync.dma_start(lnb_r[:], lnb[:])

        xt_sb = []
        for k in range(4):
            t = acts.tile([128, W], F32R, tag=f"xt{k}")
            nc.sync.dma_start(t[:], xt[k * 128:(k + 1) * 128, :])
            xt_sb.append(t)

        # ln gamma/beta broadcast to 128 partitions
        lng_bc = const.tile([128, D_IN], F32)
        lnb_bc = const.tile([128, D_IN], F32)
        for src, dst in ((lng_r, lng_bc), (lnb_r, lnb_bc)):
            p = pst.tile([128, D_IN], F32, tag="aux", bufs=1, name="p_lnbc")
            nc.tensor.matmul(p[:], ones_sb[:], src[:], start=True, stop=True)
            nc.scalar.copy(dst[:], p[:])

        # rc edge mask: zero rc1 at columns outside the sequence so the
        # second routing conv sees the same zero padding as the reference
        rcm_r = const.tile([1, 528], F32R)
        nc.sync.dma_start(rcm_r[:], rcmask[:])
        rcm_bc = const.tile([128, 528], F32R)
        for t0 in (0, 264):
            pmask = psum.tile([128, 264], F32, tag="aux", bufs=1, name="pmask")
            nc.tensor.matmul(pmask[:], ones_sb[:], rcm_r[:, t0:t0 + 264],
                             start=True, stop=True)
            nc.vector.tensor_scalar(rcm_bc[:, t0:t0 + 264], pmask[:], 0.0, None,
                                    op0=OP.add)

        # =========== shared conv1: sh1 = silu(conv9(x)), [16][128, 512] ===========
        sh1_sb = []
        for m in range(16):
            t = acts.tile([128, TPC], F32R, tag=f"sh1_{m}")
            sh1_sb.append(t)
        for mb in range(4):          # m-blocks of 4 chunks
            plist = [psum.tile([128, TPC], F32, tag="mm", name=f"psh_{mb}_{i}") for i in range(4)]
            first = True
            for tap in range(9):
                for k in range(4):
                    wsub = wstream.tile([128, 512], F32R, tag="w", name="wsub")
                    nc.sync.dma_start(
                        wsub[:], shw1t[tap, k * 128:(k + 1) * 128,
                                       mb * 512:(mb + 1) * 512])
                    for mi in range(4):
                        nc.tensor.matmul(
                            plist[mi][:], wsub[:, mi * 128:(mi + 1) * 128],
                            xt_sb[k][:, C0 + tap - 4: C0 + tap - 4 + TPC],
                            start=first, stop=(tap == 8 and k == 3))
                    first = False
            for mi in range(4):
                m = mb * 4 + mi
                nc.scalar.activation(sh1_sb[m][:], plist[mi][:], AF.Silu,
                                     bias=shb1_sb[:, m:m + 1])

        # =========== shared conv2 (k=1): sh2 psum kept for final combine ===========
        sh2_sb = []
        for mo in range(4):
            t = acts.tile([128, TPC], F32, tag=f"sh2_{mo}")
            sh2_sb.append(t)
        s2list = [psum.tile([128, TPC], F32, tag="mm", name=f"ps2_{i}") for i in range(4)]
        for k in range(16):
            wsub = wstream.tile([128, 512], F32R, tag="w", name="wsub")
            nc.sync.dma_start(wsub[:], shw2t[k * 128:(k + 1) * 128, :])
            for mo in range(4):
                nc.tensor.matmul(s2list[mo][:], wsub[:, mo * 128:(mo + 1) * 128],
                                 sh1_sb[k][:], start=(k == 0), stop=(k == 15))
        for mo in range(4):
            nc.scalar.activation(sh2_sb[mo][:], s2list[mo][:], AF.Identity,
                                 bias=shb2_sb[:, mo:mo + 1])

        # =========== routing conv1: rc1 = gelu(conv3(x, rc_w1)) ===========
        # computed over buffer cols [56, 584) -> rc1 col c == buffer col 56+c
        RC_LO, RC_W = 56, 528
        with tc.tile_pool(name=R + "routing", bufs=1) as rpool:
            rcw2_sb = {}
            for tap in range(3):
                for k in range(4):
                    t = rpool.tile([128, E], F32R, tag=f"rcw2_{tap}_{k}")
                    nc.sync.dma_start(t[:], rcw2t[tap, k * 128:(k + 1) * 128, :])
                    rcw2_sb[tap, k] = t

            rc1_sb = []
            for m in range(4):
                t = rpool.tile([128, RC_W], F32R, tag=f"rc1_{m}")
                rc1_sb.append(t)
            for t0, tw in ((0, 264), (264, 264)):
                plist = [psum.tile([128, tw], F32, tag="mm", name=f"prc_{t0}_{i}")
                         for i in range(4)]
                for tap in range(3):
                    for k in range(4):
                        wsub = wstream.tile([128, 512], F32R, tag="w", name="wsub")
                        nc.sync.dma_start(wsub[:], rcw1t[tap, k * 128:(k + 1) * 128, :])
                        for m in range(4):
                            nc.tensor.matmul(
                                plist[m][:], wsub[:, m * 128:(m + 1) * 128],
                                xt_sb[k][:, RC_LO + t0 + tap - 1: RC_LO + t0 + tap - 1 + tw],
                                start=(tap == 0 and k == 0), stop=(tap == 2 and k == 3))
                for m in range(4):
                    nc.scalar.activation(rc1_sb[m][:, t0:t0 + tw], plist[m][:], AF.Gelu,
                                         bias=rcb1_sb[:, m:m + 1])
                    nc.vector.tensor_tensor(rc1_sb[m][:, t0:t0 + tw],
                                            rc1_sb[m][:, t0:t0 + tw],
                                            rcm_bc[:, t0:t0 + tw], op=OP.mult)

            # ======= routing conv2 -> logits [E, 512] (center tokens) =======
            lgp = pst.tile([E, TPC], F32, tag="aux", bufs=1, name="lgp")
            first = True
            for tap in range(3):
                for k in range(4):
                    nc.tensor.matmul(
                        lgp[:], rcw2_sb[tap, k][:],
                        rc1_sb[k][:, (C0 - RC_LO) + tap - 1: (C0 - RC_LO) + tap - 1 + TPC],
                        start=first, stop=(tap == 2 and k == 3))
                    first = False
            lg_sb = rpool.tile([E, TPC], F32, tag="lg")
            nc.scalar.activation(lg_sb[:], lgp[:], AF.Identity, bias=rcb2_sb[:])

            # ======= top-2 gating -> gatesT [E, 512] (f32r) =======
            gatesT = acts.tile([E, TPC], F32R)
            for tt in range(4):
                tp = pst.tile([128, E], F32, tag="aux", bufs=1, name="tp")
                nc.tensor.transpose(tp[:], lg_sb[:, tt * 128:(tt + 1) * 128],
                                    id_sb[0:E, 0:E])
                lT = scratch.tile([128, E], F32, tag="lT")
                nc.scalar.copy(lT[:], tp[:])
                m1 = scratch.tile([128, 1], F32, tag="m1")
                nc.vector.reduce_max(m1[:], lT[:], axis=AX.X)
                mask1 = scratch.tile([128, E], F32, tag="mask1")
                nc.vector.tensor_scalar(mask1[:], lT[:], m1[:], None, op0=OP.is_equal)
                lmask = scratch.tile([128, E], F32, tag="lmask")
                nc.vector.scalar_tensor_tensor(lmask[:], mask1[:], -1e30, lT[:],
                                               op0=OP.mult, op1=OP.add)
                m2 = scratch.tile([128, 1], F32, tag="m2")
                nc.vector.reduce_max(m2[:], lmask[:], axis=AX.X)
                mask2 = scratch.tile([128, E], F32, tag="mask2")
                nc.vector.tensor_scalar(mask2[:], lmask[:], m2[:], None, op0=OP.is_equal)
                d = scratch.tile([128, 1], F32, tag="d")
                nc.vector.tensor_scalar(d[:], m2[:], m1[:], None, op0=OP.subtract)
                e_ = scratch.tile([128, 1], F32, tag="e_")
                nc.scalar.activation(e_[:], d[:], AF.Exp)
                ope = scratch.tile([128, 1], F32, tag="ope")
                nc.vector.tensor_scalar(ope[:], e_[:], 1.0, None, op0=OP.add)
                g1 = scratch.tile([128, 1], F32, tag="g1")
                nc.vector.reciprocal(g1[:], ope[:])
                g2 = scratch.tile([128, 1], F32, tag="g2")
                nc.vector.tensor_scalar(g2[:], g1[:], -1.0, 1.0, op0=OP.mult, op1=OP.add)
                t1 = scratch.tile([128, E], F32, tag="t1")
                nc.vector.tensor_scalar(t1[:], mask1[:], g1[:], None, op0=OP.mult)
                gt = scratch.tile([128, E], F32, tag="gt")
                nc.vector.scalar_tensor_tensor(gt[:], mask2[:], g2[:], t1[:],
                                               op0=OP.mult, op1=OP.add)
                gp = pst.tile([E, 128], F32, tag="aux", bufs=1, name="gp")
                nc.tensor.transpose(gp[:], gt[:], id_sb[:])
                nc.vector.tensor_scalar(gatesT[:, tt * 128:(tt + 1) * 128], gp[:],
                                        0.0, None, op0=OP.add)

        # =========== experts (dense, gated accumulate into y_acc) ===========
        y_acc = [acts.tile([128, TPC], F32, tag=f"y{mo}", name=f"y_acc_{mo}") for mo in range(4)]
        for e in range(E):
            b1 = scratch.tile([128, 16], F32, tag="b1")
            nc.sync.dma_start(b1[:], eb1p[e])
            bce = scratch.tile([128, 4], F32, tag="bce")
            nc.sync.dma_start(bce[:], ebcp[e])

            h_sb = []
            for mb in range(4):
                plist = [psum.tile([128, TPC], F32, tag="mm", name=f"ph_{e}_{mb}_{i}") for i in range(4)]
                for k in range(4):
                    wsub = wstream.tile([128, 512], F32R, tag="w", name="wsub")
                    nc.sync.dma_start(
                        wsub[:], ew1[e, k * 128:(k + 1) * 128,
                                     mb * 512:(mb + 1) * 512])
                    for mi in range(4):
                        nc.tensor.matmul(
                            plist[mi][:], wsub[:, mi * 128:(mi + 1) * 128],
                            xt_sb[k][:, C0:C0 + TPC],
                            start=(k == 0), stop=(k == 3))
                for mi in range(4):
                    m = mb * 4 + mi
                    # elu(v) = max(v, min(exp(v), 1) - 1), v = h + b1
                    eh = scratch.tile([128, TPC], F32, tag="he")
                    nc.scalar.activation(eh[:], plist[mi][:], AF.Exp,
                                         bias=b1[:, m:m + 1])
                    em = scratch.tile([128, TPC], F32, tag="hm")
                    nc.vector.tensor_scalar(em[:], eh[:], 1.0, -1.0,
                                            op0=OP.min, op1=OP.add)
                    h1 = hpool.tile([128, TPC], F32R, tag="h")
                    nc.vector.scalar_tensor_tensor(h1[:], plist[mi][:],
                                                   b1[:, m:m + 1], em[:],
                                                   op0=OP.add, op1=OP.max)
                    h_sb.append(h1)

            # broadcast this expert's gate row
            bcp = pst.tile([128, TPC], F32, tag="aux", bufs=1, name="bcp")
            nc.tensor.matmul(bcp[:], sel_sb[:, e * 128:(e + 1) * 128], gatesT[:],
                             start=True, stop=True)
            bc_sb = scratch.tile([128, TPC], F32, tag="bc")
            nc.scalar.copy(bc_sb[:], bcp[:])

            elist = [psum.tile([128, TPC], F32, tag="mm", name=f"pe_{e}_{i}") for i in range(4)]
            for k in range(16):
                wsub = wstream.tile([128, 512], F32R, tag="w", name="wsub")
                nc.sync.dma_start(wsub[:], ew2[e, k * 128:(k + 1) * 128, :])
                for mo in range(4):
                    nc.tensor.matmul(elist[mo][:], wsub[:, mo * 128:(mo + 1) * 128],
                                     h_sb[k][:], start=(k == 0), stop=(k == 15))
            for mo in range(4):
                # (eo + (e_b2 - colsum)) * gate, accumulated into y_acc
                if e == 0:
                    nc.vector.scalar_tensor_tensor(
                        y_acc[mo][:], elist[mo][:], bce[:, mo:mo + 1], bc_sb[:],
                        op0=OP.add, op1=OP.mult)
                else:
                    yt = scratch.tile([128, TPC], F32, tag="yt")
                    nc.vector.scalar_tensor_tensor(
                        yt[:], elist[mo][:], bce[:, mo:mo + 1], bc_sb[:],
                        op0=OP.add, op1=OP.mult)
                    nc.vector.tensor_tensor(y_acc[mo][:], y_acc[mo][:], yt[:],
                                            op=OP.add)

        # =========== z = x + y + sh2 ; transpose ; layernorm ; out ===========
        z_sb = []
        for mo in range(4):
            z = acts.tile([128, TPC], F32, tag=f"z{mo}")
            nc.vector.tensor_tensor(z[:], y_acc[mo][:], sh2_sb[mo][:], op=OP.add)
            nc.vector.tensor_tensor(z[:], z[:],
                                    xt_sb[mo][:, C0:C0 + TPC].bitcast(F32), op=OP.add)
            z_sb.append(z)

        for tt in range(4):
            zT = scratch.tile([128, D_IN], F32, tag="zT")
            for mo in range(4):
                ztp = pst.tile([128, 128], F32, tag="aux", bufs=1, name="ztp")
                nc.tensor.transpose(ztp[:], z_sb[mo][:, tt * 128:(tt + 1) * 128],
                                    id_sb[:])
                nc.scalar.copy(zT[:, mo * 128:(mo + 1) * 128], ztp[:])
            srow = scratch.tile([128, 1], F32, tag="srow")
            nc.vector.reduce_sum(srow[:], zT[:], axis=AX.X)
            nmean = scratch.tile([128, 1], F32, tag="nmean")
            nc.vector.tensor_scalar(nmean[:], srow[:], -1.0 / D_IN, None, op0=OP.mult)
            zc = scratch.tile([128, D_IN], F32, tag="zc")
            nc.vector.tensor_scalar(zc[:], zT[:], nmean[:], None, op0=OP.add)
            sq = scratch.tile([128, D_IN], F32, tag="sq")
            ssq = scratch.tile([128, 1], F32, tag="ssq")
            nc.scalar.activation(sq[:], zc[:], AF.Square, accum_out=ssq[:])
            vpe = scratch.tile([128, 1], F32, tag="vpe")
            nc.vector.tensor_scalar(vpe[:], ssq[:], 1.0 / D_IN, 1e-5,
                                    op0=OP.mult, op1=OP.add)
            rinv = scratch.tile([128, 1], F32, tag="rinv")
            nc.vector.reciprocal(rinv[:], vpe[:])
            rstd = scratch.tile([128, 1], F32, tag="rstd")
            nc.scalar.activation(rstd[:], rinv[:], AF.Sqrt)
            normed = scratch.tile([128, D_IN], F32, tag="normed")
            nc.vector.tensor_scalar(normed[:], zc[:], rstd[:], None, op0=OP.mult)
            og = scratch.tile([128, D_IN], F32, tag="og")
            nc.vector.tensor_tensor(og[:], normed[:], lng_bc[:], op=OP.mult)
            out = scratch.tile([128, D_IN], F32, tag="out")
            nc.vector.tensor_tensor(out[:], og[:], lnb_bc[:], op=OP.add)
            nc.sync.dma_start(yout[tt * 128:(tt + 1) * 128, :], out[:])

    _split_multi_waits(nc)
    return nc


_CACHE = {}


def _get_nc(reps=1):
    key = f"nc{reps}"
    if key not in _CACHE:
        _CACHE[key] = _build(reps)
    return _CACHE[key]


LAST_RESULT = {}


def kernel(x, rc_w1, rc_b1, rc_w2, rc_b2, sh_w1, sh_b1, sh_w2, sh_b2,
           e_w1, e_b1, e_w2, e_b2, ln_g, ln_b, **kwargs):
    x = np.asarray(x, np.float32)
    f = lambda a: np.ascontiguousarray(np.asarray(a, np.float32))

    shared = {
        "rcw1t": f(np.asarray(rc_w1, np.float32).transpose(2, 1, 0)),
        "rcw2t": f(np.asarray(rc_w2, np.float32).transpose(2, 1, 0)),
        "shw1t": f(np.asarray(sh_w1, np.float32).transpose(2, 1, 0)),
        "shw2t": f(np.asarray(sh_w2, np.float32)[:, :, 0].T),
        "ew1": f(e_w1),
        "ew2": f(e_w2),
        "rcb1": f(np.asarray(rc_b1, np.float32).reshape(4, 128).T),
        "rcb2": f(np.asarray(rc_b2, np.float32).reshape(E, 1)),
        "shb1": f(np.asarray(sh_b1, np.float32).reshape(16, 128).T),
        "shb2": f(np.asarray(sh_b2, np.float32).reshape(4, 128).T),
        "eb1p": f(np.asarray(e_b1, np.float32).reshape(E, 16, 128).transpose(0, 2, 1)),
        "ebcp": f(np.asarray(e_b2, np.float32).reshape(E, 4, 128).transpose(0, 2, 1)),
        "lng": f(np.asarray(ln_g, np.float32).reshape(1, D_IN)),
        "lnb": f(np.asarray(ln_b, np.float32).reshape(1, D_IN)),
        "ones": np.ones((1, 128), np.float32),
        "sel": np.repeat(np.eye(E, dtype=np.float32), 128, axis=1),
        "ident": np.eye(128, dtype=np.float32),
    }

    in_maps = []
    for c in range(NCORES):
        b, j = divmod(c, T // TPC)
        lo_tok = j * TPC - HALO
        hi_tok = j * TPC + TPC + HALO
        xh = np.zeros((W, D_IN), np.float32)
        lo = max(0, lo_tok)
        hi = min(T, hi_tok)
        xh[lo - lo_tok: hi - lo_tok] = x[b, lo:hi]
        im = dict(shared)
        im["xt"] = np.ascontiguousarray(xh.T)
        rcm = np.zeros((1, 528), np.float32)
        for cidx in range(528):
            tok = j * TPC - 8 + cidx
            if 0 <= tok < T:
                rcm[0, cidx] = 1.0
        im["rcmask"] = rcm
        in_maps.append(im)

    nc = _get_nc()
    res = run_bass_kernel_spmd(nc, in_maps, core_ids=list(range(NCORES)),
                               **kwargs)
    LAST_RESULT["res"] = res

    out = np.empty((B, T, D_IN), np.float32)
    for c in range(NCORES):
        b, j = divmod(c, T // TPC)
        out[b, j * TPC:(j + 1) * TPC] = res.results[c]["yout"]
    return out



# revision 2
# speedup vs baseline: 1.2227x; 1.2227x over previous
"""Trainium2 Bass kernel for nn_MixtureOfExperts_29867202576447.

Strategy: data-parallel over tokens (8 cores x 512 tokens), dense expert
compute with top-2 gates applied as a mask. Routing convs stay float32r
(top-k decisions need the precision); shared convs run in bf16; the expert
GEMMs (the dominant FLOPs) run in fp8e4 with DoubleRow perf mode (2
contraction subtiles per instruction at 0.5 cycles/row). Power-of-2
scaling keeps fp8 operands in the normal range exactly: xq = q(x/4),
W1q = q(4 w1) so the product is unscaled; W2q = q(4 w2) with the 1/4
folded into the per-expert gate broadcast.

Self-contained: hardcodes all shapes; host-side prep only shards/pads x,
transposes + quantizes weights, and packs biases.
"""
import numpy as np
import ml_dtypes
from contextlib import ExitStack

import concourse.bass as bass
import concourse.tile as tile
import concourse.mybir as mybir
from concourse.bass_utils import run_bass_kernel_spmd

F32R = mybir.dt.float32r
F32 = mybir.dt.float32
BF16 = mybir.dt.bfloat16
FP8 = mybir.dt.float8e4
DR = mybir.MatmulPerfMode.DoubleRow
AF = mybir.ActivationFunctionType
OP = mybir.AluOpType
AX = mybir.AxisListType

D_IN, D_HID, E = 512, 2048, 10
B, T = 2, 2048
TPC = 512          # tokens per core
HALO = 64          # halo columns on each side of the token window
W = TPC + 2 * HALO  # 640 buffer columns per core
NCORES = 8

_ctr = [0]


def _split_multi_waits(nc, max_waits=1):
    """walrus here accepts one sync-wait per instruction; hoist extras onto
    same-engine NoOps placed immediately before the instruction."""
    n = 0
    for f in nc.m.functions:
        for bb in f.blocks:
            out = []
            changed = False
            for ins in bb.instructions:
                si = getattr(ins, "sync_info", None)
                waits = list(si.on_wait) if (si is not None and si.on_wait) else []
                if len(waits) > max_waits:
                    for w in waits[:-max_waits]:
                        _ctr[0] += 1
                        nop = mybir.InstNoOp(
                            name=f"I-waitsplit-{_ctr[0]}", engine=ins.engine,
                            ins=[], outs=[])
                        nop.sync_info = mybir.SyncInfo(on_wait=[w], on_update=[])
                        nc.register_instruction(nop)
                        out.append(nop)
                    si.on_wait = waits[-max_waits:]
                    changed = True
                    n += 1
                out.append(ins)
            if changed:
                bb.instructions = out
    return n


def _build(reps=1):
    nc = bass.Bass(trn_type="TRN2")

    # ---------------- DRAM I/O ----------------
    xt = nc.dram_tensor("xt", [D_IN, W], F32R, kind="ExternalInput")
    xb = nc.dram_tensor("xb", [D_IN, W], BF16, kind="ExternalInput")
    xq = nc.dram_tensor("xq", [2, 128, 2, W], FP8, kind="ExternalInput")
    rcw1t = nc.dram_tensor("rcw1t", [3, D_IN, D_IN], F32R, kind="ExternalInput")
    rcw2t = nc.dram_tensor("rcw2t", [3, D_IN, E], F32R, kind="ExternalInput")
    shw1t = nc.dram_tensor("shw1t", [9, D_IN, D_HID], BF16, kind="ExternalInput")
    shw2t = nc.dram_tensor("shw2t", [D_HID, D_IN], BF16, kind="ExternalInput")
    ew1q = nc.dram_tensor("ew1q", [E, 2, 128, 2, D_HID], FP8, kind="ExternalInput")
    ew2q = nc.dram_tensor("ew2q", [E, 8, 128, 2, D_IN], FP8, kind="ExternalInput")
    rcb1 = nc.dram_tensor("rcb1", [128, 4], F32, kind="ExternalInput")
    rcb2 = nc.dram_tensor("rcb2", [E, 1], F32, kind="ExternalInput")
    shb1 = nc.dram_tensor("shb1", [128, 16], F32, kind="ExternalInput")
    shb2 = nc.dram_tensor("shb2", [128, 4], F32, kind="ExternalInput")
    eb1p = nc.dram_tensor("eb1p", [E, 128, 16], F32, kind="ExternalInput")
    ebcp = nc.dram_tensor("ebcp", [E, 128, 4], F32, kind="ExternalInput")
    lng = nc.dram_tensor("lng", [1, D_IN], F32R, kind="ExternalInput")
    lnb = nc.dram_tensor("lnb", [1, D_IN], F32R, kind="ExternalInput")
    ones = nc.dram_tensor("ones", [1, 128], F32R, kind="ExternalInput")
    sel = nc.dram_tensor("sel", [E, E * 128], F32R, kind="ExternalInput")
    ident = nc.dram_tensor("ident", [128, 128], F32, kind="ExternalInput")
    rcmask = nc.dram_tensor("rcmask", [1, 528], F32R, kind="ExternalInput")
    yout = nc.dram_tensor("yout", [TPC, D_IN], F32, kind="ExternalOutput")

    C0 = HALO            # buffer col of first center token
    with tile.TileContext(nc) as tc:
      for rep in range(reps):
       with ExitStack() as ctx:
        R = f"r{rep}_"
        const = ctx.enter_context(tc.tile_pool(name=R + "const", bufs=1))
        acts = ctx.enter_context(tc.tile_pool(name=R + "acts", bufs=1))
        wstream = ctx.enter_context(tc.tile_pool(name=R + "wstream", bufs=10))
        scratch = ctx.enter_context(tc.tile_pool(name=R + "scratch", bufs=2))
        hpool = ctx.enter_context(tc.tile_pool(name=R + "hpool", bufs=10))
        psum = ctx.enter_context(tc.tile_pool(name=R + "psum", bufs=7, space="PSUM"))
        pst = psum

        # ---------------- constants / x ----------------
        id_sb = const.tile([128, 128], F32)
        nc.sync.dma_start(id_sb[:], ident[:])
        ones_sb = const.tile([1, 128], F32R)
        nc.sync.dma_start(ones_sb[:], ones[:])
        sel_sb = const.tile([E, E * 128], F32R)
        nc.sync.dma_start(sel_sb[:], sel[:])
        rcb1_sb = const.tile([128, 4], F32)
        nc.sync.dma_start(rcb1_sb[:], rcb1[:])
        rcb2_sb = const.tile([E, 1], F32)
        nc.sync.dma_start(rcb2_sb[:], rcb2[:])
        shb1_sb = const.tile([128, 16], F32)
        nc.sync.dma_start(shb1_sb[:], shb1[:])
        shb2_sb = const.tile([128, 4], F32)
        nc.sync.dma_start(shb2_sb[:], shb2[:])
        lng_r = const.tile([1, D_IN], F32R)
        nc.sync.dma_start(lng_r[:], lng[:])
        lnb_r = const.tile([1, D_IN], F32R)
        nc.sync.dma_start(lnb_r[:], lnb[:])

        xt_sb = []
        for k in range(4):
            t = acts.tile([128, W], F32R, tag=f"xt{k}")
            nc.sync.dma_start(t[:], xt[k * 128:(k + 1) * 128, :])
            xt_sb.append(t)
        xb_sb = []
        for k in range(4):
            t = acts.tile([128, W], BF16, tag=f"xb{k}")
            nc.sync.dma_start(t[:], xb[k * 128:(k + 1) * 128, :])
            xb_sb.append(t)
        xq_sb = []
        for p in range(2):
            t = acts.tile([128, 2, W], FP8, tag=f"xq{p}")
            nc.sync.dma_start(t[:], xq[p])
            xq_sb.append(t)

        # ln gamma/beta broadcast to 128 partitions
        lng_bc = const.tile([128, D_IN], F32)
        lnb_bc = const.tile([128, D_IN], F32)
        for src, dst in ((lng_r, lng_bc), (lnb_r, lnb_bc)):
            p = pst.tile([128, D_IN], F32, tag="aux", bufs=1, name="p_lnbc")
            nc.tensor.matmul(p[:], ones_sb[:], src[:], start=True, stop=True)
            nc.scalar.copy(dst[:], p[:])

        # rc edge mask: zero rc1 at columns outside the sequence so the
        # second routing conv sees the same zero padding as the reference
        rcm_r = const.tile([1, 528], F32R)
        nc.sync.dma_start(rcm_r[:], rcmask[:])
        rcm_bc = const.tile([128, 528], F32R)
        for t0 in (0, 264):
            pmask = psum.tile([128, 264], F32, tag="aux", bufs=1, name="pmask")
            nc.tensor.matmul(pmask[:], ones_sb[:], rcm_r[:, t0:t0 + 264],
                             start=True, stop=True)
            nc.vector.tensor_scalar(rcm_bc[:, t0:t0 + 264], pmask[:], 0.0, None,
                                    op0=OP.add)

        # =========== shared conv1: sh1 = silu(conv9(x)), bf16 [16][128, 512] ======
        sh1_sb = []
        for m in range(16):
            t = acts.tile([128, TPC], BF16, tag=f"sh1_{m}")
            sh1_sb.append(t)
        for mb in range(4):          # m-blocks of 4 chunks
            plist = [psum.tile([128, TPC], F32, tag="mm", name=f"psh_{mb}_{i}") for i in range(4)]
            first = True
            for tap in range(9):
                for k in range(4):
                    wsub = wstream.tile([128, 512], BF16, tag="wb", name="wsubb")
                    nc.sync.dma_start(
                        wsub[:], shw1t[tap, k * 128:(k + 1) * 128,
                                       mb * 512:(mb + 1) * 512])
                    for mi in range(4):
                        nc.tensor.matmul(
                            plist[mi][:], wsub[:, mi * 128:(mi + 1) * 128],
                            xb_sb[k][:, C0 + tap - 4: C0 + tap - 4 + TPC],
                            start=first, stop=(tap == 8 and k == 3))
                    first = False
            for mi in range(4):
                m = mb * 4 + mi
                nc.scalar.activation(sh1_sb[m][:], plist[mi][:], AF.Silu,
                                     bias=shb1_sb[:, m:m + 1])

        # =========== shared conv2 (k=1, bf16): sh2 kept f32 for final combine =====
        sh2_sb = []
        for mo in range(4):
            t = acts.tile([128, TPC], F32, tag=f"sh2_{mo}")
            sh2_sb.append(t)
        s2list = [psum.tile([128, TPC], F32, tag="mm", name=f"ps2_{i}") for i in range(4)]
        for k in range(16):
            wsub = wstream.tile([128, 512], BF16, tag="wb", name="wsubb")
            nc.sync.dma_start(wsub[:], shw2t[k * 128:(k + 1) * 128, :])
            for mo in range(4):
                nc.tensor.matmul(s2list[mo][:], wsub[:, mo * 128:(mo + 1) * 128],
                                 sh1_sb[k][:], start=(k == 0), stop=(k == 15))
        for mo in range(4):
            nc.scalar.activation(sh2_sb[mo][:], s2list[mo][:], AF.Identity,
                                 bias=shb2_sb[:, mo:mo + 1])

        # =========== routing conv1: rc1 = gelu(conv3(x, rc_w1)) ===========
        # computed over buffer cols [56, 584) -> rc1 col c == buffer col 56+c
        RC_LO, RC_W = 56, 528
        with tc.tile_pool(name=R + "routing", bufs=1) as rpool:
            rcw2_sb = {}
            for tap in range(3):
                for k in range(4):
                    t = rpool.tile([128, E], F32R, tag=f"rcw2_{tap}_{k}")
                    nc.sync.dma_start(t[:], rcw2t[tap, k * 128:(k + 1) * 128, :])
                    rcw2_sb[tap, k] = t

            rc1_sb = []
            for m in range(4):
                t = rpool.tile([128, RC_W], F32R, tag=f"rc1_{m}")
                rc1_sb.append(t)
            for t0, tw in ((0, 264), (264, 264)):
                plist = [psum.tile([128, tw], F32, tag="mm", name=f"prc_{t0}_{i}")
                         for i in range(4)]
                for tap in range(3):
                    for k in range(4):
                        wsub = wstream.tile([128, 512], F32R, tag="w", name="wsub")
                        nc.sync.dma_start(wsub[:], rcw1t[tap, k * 128:(k + 1) * 128, :])
                        for m in range(4):
                            nc.tensor.matmul(
                                plist[m][:], wsub[:, m * 128:(m + 1) * 128],
                                xt_sb[k][:, RC_LO + t0 + tap - 1: RC_LO + t0 + tap - 1 + tw],
                                start=(tap == 0 and k == 0), stop=(tap == 2 and k == 3))
                for m in range(4):
                    nc.scalar.activation(rc1_sb[m][:, t0:t0 + tw], plist[m][:], AF.Gelu,
                                         bias=rcb1_sb[:, m:m + 1])
                    nc.vector.tensor_tensor(rc1_sb[m][:, t0:t0 + tw],
                                            rc1_sb[m][:, t0:t0 + tw],
                                            rcm_bc[:, t0:t0 + tw], op=OP.mult)

            # ======= routing conv2 -> logits [E, 512] (center tokens) =======
            lgp = pst.tile([E, TPC], F32, tag="aux", bufs=1, name="lgp")
            first = True
            for tap in range(3):
                for k in range(4):
                    nc.tensor.matmul(
                        lgp[:], rcw2_sb[tap, k][:],
                        rc1_sb[k][:, (C0 - RC_LO) + tap - 1: (C0 - RC_LO) + tap - 1 + TPC],
                        start=first, stop=(tap == 2 and k == 3))
                    first = False
            lg_sb = rpool.tile([E, TPC], F32, tag="lg")
            nc.scalar.activation(lg_sb[:], lgp[:], AF.Identity, bias=rcb2_sb[:])

            # ======= top-2 gating -> gatesT [E, 512] (f32r) =======
            gatesT = acts.tile([E, TPC], F32R)
            for tt in range(4):
                tp = pst.tile([128, E], F32, tag="aux", bufs=1, name="tp")
                nc.tensor.transpose(tp[:], lg_sb[:, tt * 128:(tt + 1) * 128],
                                    id_sb[0:E, 0:E])
                lT = scratch.tile([128, E], F32, tag="lT")
                nc.scalar.copy(lT[:], tp[:])
                m1 = scratch.tile([128, 1], F32, tag="m1")
                nc.vector.reduce_max(m1[:], lT[:], axis=AX.X)
                mask1 = scratch.tile([128, E], F32, tag="mask1")
                nc.vector.tensor_scalar(mask1[:], lT[:], m1[:], None, op0=OP.is_equal)
                lmask = scratch.tile([128, E], F32, tag="lmask")
                nc.vector.scalar_tensor_tensor(lmask[:], mask1[:], -1e30, lT[:],
                                               op0=OP.mult, op1=OP.add)
                m2 = scratch.tile([128, 1], F32, tag="m2")
                nc.vector.reduce_max(m2[:], lmask[:], axis=AX.X)
                mask2 = scratch.tile([128, E], F32, tag="mask2")
                nc.vector.tensor_scalar(mask2[:], lmask[:], m2[:], None, op0=OP.is_equal)
                d = scratch.tile([128, 1], F32, tag="d")
                nc.vector.tensor_scalar(d[:], m2[:], m1[:], None, op0=OP.subtract)
                e_ = scratch.tile([128, 1], F32, tag="e_")
                nc.scalar.activation(e_[:], d[:], AF.Exp)
                ope = scratch.tile([128, 1], F32, tag="ope")
                nc.vector.tensor_scalar(ope[:], e_[:], 1.0, None, op0=OP.add)
                g1 = scratch.tile([128, 1], F32, tag="g1")
                nc.vector.reciprocal(g1[:], ope[:])
                g2 = scratch.tile([128, 1], F32, tag="g2")
                nc.vector.tensor_scalar(g2[:], g1[:], -1.0, 1.0, op0=OP.mult, op1=OP.add)
                t1 = scratch.tile([128, E], F32, tag="t1")
                nc.vector.tensor_scalar(t1[:], mask1[:], g1[:], None, op0=OP.mult)
                gt = scratch.tile([128, E], F32, tag="gt")
                nc.vector.scalar_tensor_tensor(gt[:], mask2[:], g2[:], t1[:],
                                               op0=OP.mult, op1=OP.add)
                gp = pst.tile([E, 128], F32, tag="aux", bufs=1, name="gp")
                nc.tensor.transpose(gp[:], gt[:], id_sb[:])
                nc.vector.tensor_scalar(gatesT[:, tt * 128:(tt + 1) * 128], gp[:],
                                        0.0, None, op0=OP.add)

        # =========== experts (dense fp8 DoubleRow, gated accumulate) ===========
        y_acc = [acts.tile([128, TPC], F32, tag=f"y{mo}", name=f"y_acc_{mo}") for mo in range(4)]
        for e in range(E):
            b1 = scratch.tile([128, 16], F32, tag="b1")
            nc.sync.dma_start(b1[:], eb1p[e])
            bce = scratch.tile([128, 4], F32, tag="bce")
            nc.sync.dma_start(bce[:], ebcp[e])

            h_sb = [hpool.tile([128, 2, TPC], FP8, tag="h", name=f"h_{e}_{i}")
                    for i in range(8)]
            for mb in range(4):
                plist = [psum.tile([128, TPC], F32, tag="mm", name=f"ph_{e}_{mb}_{i}") for i in range(4)]
                for p in range(2):
                    wsub = wstream.tile([128, 2, 512], FP8, tag="w8", name="wsub8")
                    nc.sync.dma_start(
                        wsub[:], ew1q[e, p, :, :, mb * 512:(mb + 1) * 512])
                    for mi in range(4):
                        nc.tensor.matmul(
                            plist[mi][:], wsub[:, :, mi * 128:(mi + 1) * 128],
                            xq_sb[p][:, :, C0:C0 + TPC],
                            start=(p == 0), stop=(p == 1), perf_mode=DR)
                for mi in range(4):
                    m = mb * 4 + mi
                    # elu(v) = max(v, min(exp(v), 1) - 1), v = h + b1
                    eh = scratch.tile([128, TPC], BF16, tag="he")
                    nc.scalar.activation(eh[:], plist[mi][:], AF.Exp,
                                         bias=b1[:, m:m + 1])
                    em = scratch.tile([128, TPC], BF16, tag="hm")
                    nc.vector.tensor_scalar(em[:], eh[:], 1.0, -1.0,
                                            op0=OP.min, op1=OP.add)
                    nc.vector.scalar_tensor_tensor(h_sb[m // 2][:, m % 2, :],
                                                   plist[mi][:],
                                                   b1[:, m:m + 1], em[:],
                                                   op0=OP.add, op1=OP.max)

            # broadcast this expert's gate row, folding in the 1/4 W2 scale
            bcp = pst.tile([128, TPC], F32, tag="aux", bufs=1, name="bcp")
            nc.tensor.matmul(bcp[:], sel_sb[:, e * 128:(e + 1) * 128], gatesT[:],
                             start=True, stop=True)
            bc_sb = scratch.tile([128, TPC], F32, tag="bc")
            nc.scalar.activation(bc_sb[:], bcp[:], AF.Copy, scale=0.25)

            elist = [psum.tile([128, TPC], F32, tag="mm", name=f"pe_{e}_{i}") for i in range(4)]
            for q in range(8):
                wsub = wstream.tile([128, 2, 512], FP8, tag="w8", name="wsub8")
                nc.sync.dma_start(wsub[:], ew2q[e, q])
                for mo in range(4):
                    nc.tensor.matmul(elist[mo][:], wsub[:, :, mo * 128:(mo + 1) * 128],
                                     h_sb[q][:], start=(q == 0), stop=(q == 7),
                                     perf_mode=DR)
            for mo in range(4):
                # (eo + (4 e_b2 - colsum)) * gate/4, accumulated into y_acc
                if e == 0:
                    nc.vector.scalar_tensor_tensor(
                        y_acc[mo][:], elist[mo][:], bce[:, mo:mo + 1], bc_sb[:],
                        op0=OP.add, op1=OP.mult)
                else:
                    yt = scratch.tile([128, TPC], F32, tag="yt")
                    nc.vector.scalar_tensor_tensor(
                        yt[:], elist[mo][:], bce[:, mo:mo + 1], bc_sb[:],
                        op0=OP.add, op1=OP.mult)
                    nc.vector.tensor_tensor(y_acc[mo][:], y_acc[mo][:], yt[:],
                                            op=OP.add)

        # =========== z = x + y + sh2 ; transpose ; layernorm ; out ===========
        z_sb = []
        for mo in range(4):
            z = acts.tile([128, TPC], F32, tag=f"z{mo}")
            nc.vector.tensor_tensor(z[:], y_acc[mo][:], sh2_sb[mo][:], op=OP.add)
            nc.vector.tensor_tensor(z[:], z[:],
                                    xt_sb[mo][:, C0:C0 + TPC].bitcast(F32), op=OP.add)
            z_sb.append(z)

        for tt in range(4):
            zT = scratch.tile([128, D_IN], F32, tag="zT")
            for mo in range(4):
                ztp = pst.tile([128, 128], F32, tag="aux", bufs=1, name="ztp")
                nc.tensor.transpose(ztp[:], z_sb[mo][:, tt * 128:(tt + 1) * 128],
                                    id_sb[:])
                nc.scalar.copy(zT[:, mo * 128:(mo + 1) * 128], ztp[:])
            srow = scratch.tile([128, 1], F32, tag="srow")
            nc.vector.reduce_sum(srow[:], zT[:], axis=AX.X)
            nmean = scratch.tile([128, 1], F32, tag="nmean")
            nc.vector.tensor_scalar(nmean[:], srow[:], -1.0 / D_IN, None, op0=OP.mult)
            zc = scratch.tile([128, D_IN], F32, tag="zc")
            nc.vector.tensor_scalar(zc[:], zT[:], nmean[:], None, op0=OP.add)
            sq = scratch.tile([128, D_IN], F32, tag="sq")
            ssq = scratch.tile([128, 1], F32, tag="ssq")
            nc.scalar.activation(sq[:], zc[:], AF.Square, accum_out=ssq[:])
            vpe = scratch.tile([128, 1], F32, tag="vpe")
            nc.vector.tensor_scalar(vpe[:], ssq[:], 1.0 / D_IN, 1e-5,
                                    op0=OP.mult, op1=OP.add)
            rinv = scratch.tile([128, 1], F32, tag="rinv")
            nc.vector.reciprocal(rinv[:], vpe[:])
            rstd = scratch.tile([128, 1], F32, tag="rstd")
            nc.scalar.activation(rstd[:], rinv[:], AF.Sqrt)
            normed = scratch.tile([128, D_IN], F32, tag="normed")
            nc.vector.tensor_scalar(normed[:], zc[:], rstd[:], None, op0=OP.mult)
            og = scratch.tile([128, D_IN], F32, tag="og")
            nc.vector.tensor_tensor(og[:], normed[:], lng_bc[:], op=OP.mult)
            out = scratch.tile([128, D_IN], F32, tag="out")
            nc.vector.tensor_tensor(out[:], og[:], lnb_bc[:], op=OP.add)
            nc.sync.dma_start(yout[tt * 128:(tt + 1) * 128, :], out[:])

    _split_multi_waits(nc)
    return nc


_CACHE = {}


def _get_nc(reps=1):
    key = f"nc{reps}"
    if key not in _CACHE:
        _CACHE[key] = _build(reps)
    return _CACHE[key]


LAST_RESULT = {}


def _fp8(a):
    return np.clip(np.asarray(a, np.float32), -240, 240).astype(ml_dtypes.float8_e4m3)


def _bf16(a):
    return np.asarray(a, np.float32).astype(ml_dtypes.bfloat16)


def kernel(x, rc_w1, rc_b1, rc_w2, rc_b2, sh_w1, sh_b1, sh_w2, sh_b2,
           e_w1, e_b1, e_w2, e_b2, ln_g, ln_b, **kwargs):
    x = np.asarray(x, np.float32)
    f = lambda a: np.ascontiguousarray(np.asarray(a, np.float32))

    # expert weights: fp8 DoubleRow layout [E, kpair, 128, 2, out]
    ew1 = np.asarray(e_w1, np.float32)          # [E, 512, 2048]
    ew1q = _fp8(np.ascontiguousarray(
        (4.0 * ew1).reshape(E, 2, 2, 128, D_HID).transpose(0, 1, 3, 2, 4)))
    ew2 = np.asarray(e_w2, np.float32)          # [E, 2048, 512]
    ew2q = _fp8(np.ascontiguousarray(
        (4.0 * ew2).reshape(E, 8, 2, 128, D_IN).transpose(0, 1, 3, 2, 4)))

    shared = {
        "rcw1t": f(np.asarray(rc_w1, np.float32).transpose(2, 1, 0)),
        "rcw2t": f(np.asarray(rc_w2, np.float32).transpose(2, 1, 0)),
        "shw1t": np.ascontiguousarray(
            _bf16(np.asarray(sh_w1, np.float32).transpose(2, 1, 0))),
        "shw2t": np.ascontiguousarray(
            _bf16(np.asarray(sh_w2, np.float32)[:, :, 0].T)),
        "ew1q": ew1q,
        "ew2q": ew2q,
        "rcb1": f(np.asarray(rc_b1, np.float32).reshape(4, 128).T),
        "rcb2": f(np.asarray(rc_b2, np.float32).reshape(E, 1)),
        "shb1": f(np.asarray(sh_b1, np.float32).reshape(16, 128).T),
        "shb2": f(np.asarray(sh_b2, np.float32).reshape(4, 128).T),
        "eb1p": f(np.asarray(e_b1, np.float32).reshape(E, 16, 128).transpose(0, 2, 1)),
        "ebcp": f(4.0 * np.asarray(e_b2, np.float32).reshape(E, 4, 128).transpose(0, 2, 1)),
        "lng": f(np.asarray(ln_g, np.float32).reshape(1, D_IN)),
        "lnb": f(np.asarray(ln_b, np.float32).reshape(1, D_IN)),
        "ones": np.ones((1, 128), np.float32),
        "sel": np.repeat(np.eye(E, dtype=np.float32), 128, axis=1),
        "ident": np.eye(128, dtype=np.float32),
    }

    in_maps = []
    for c in range(NCORES):
        b, j = divmod(c, T // TPC)
        lo_tok = j * TPC - HALO
        hi_tok = j * TPC + TPC + HALO
        xh = np.zeros((W, D_IN), np.float32)
        lo = max(0, lo_tok)
        hi = min(T, hi_tok)
        xh[lo - lo_tok: hi - lo_tok] = x[b, lo:hi]
        im = dict(shared)
        xhT = np.ascontiguousarray(xh.T)            # [512, W]
        im["xt"] = xhT
        im["xb"] = np.ascontiguousarray(_bf16(xhT))
        im["xq"] = _fp8(np.ascontiguousarray(
            (xhT / 4.0).reshape(2, 2, 128, W).transpose(0, 2, 1, 3)))
        rcm = np.zeros((1, 528), np.float32)
        for cidx in range(528):
            tok = j * TPC - 8 + cidx
            if 0 <= tok < T:
                rcm[0, cidx] = 1.0
        im["rcmask"] = rcm
        in_maps.append(im)

    nc = _get_nc()
    res = run_bass_kernel_spmd(nc, in_maps, core_ids=list(range(NCORES)),
                               **kwargs)
    LAST_RESULT["res"] = res

    out = np.empty((B, T, D_IN), np.float32)
    for c in range(NCORES):
        b, j = divmod(c, T // TPC)
        out[b, j * TPC:(j + 1) * TPC] = res.results[c]["yout"]
    return out


# revision 6
# speedup vs baseline: 1.5171x; 1.2407x over previous
"""Trainium2 Bass kernel for nn_MixtureOfExperts_29867202576447.

Strategy: data-parallel over tokens (8 cores x 512 tokens), dense expert
compute with top-2 gates applied as a mask. Routing convs stay float32r
(top-k decisions need the precision); shared convs run in bf16; the expert
GEMMs (the dominant FLOPs) run in fp8e4 with DoubleRow perf mode. Power-of-2
scaling keeps fp8 operands in the normal range exactly: xq = q(x/4),
W1q = q(4 w1) so the product is unscaled; W2q = q(4 w2) with the 1/4
folded into the per-expert gate broadcast.

Schedule: routing+gating first, then the shared conv1 tap-blocks are
interleaved into the expert loop so the PE has work while the ELU chain
(ACT exp -> Pool min/add -> DVE max) drains each expert's PSUM tiles.
PSUM groups are 2 banks each so expert W1, expert W2 and a shared-conv
block can be in flight at once. DMAs are batched (one transfer per expert
weight matrix) because the sync-engine sequencer bottlenecks on per-DMA
issue overhead.
"""
import numpy as np
import ml_dtypes
from contextlib import ExitStack

import concourse.bass as bass
import concourse.tile as tile
import concourse.mybir as mybir
from concourse.bass_utils import run_bass_kernel_spmd

F32R = mybir.dt.float32r
F32 = mybir.dt.float32
BF16 = mybir.dt.bfloat16
FP8 = mybir.dt.float8e4
DR = mybir.MatmulPerfMode.DoubleRow
AF = mybir.ActivationFunctionType
OP = mybir.AluOpType
AX = mybir.AxisListType

D_IN, D_HID, E = 512, 2048, 10
B, T = 2, 2048
TPC = 512          # tokens per core
HALO = 64          # halo columns on each side of the token window
W = TPC + 2 * HALO  # 640 buffer columns per core
NCORES = 8

_ctr = [0]


def _split_multi_waits(nc, max_waits=1):
    """walrus here accepts one sync-wait per instruction; hoist extras onto
    same-engine NoOps placed immediately before the instruction."""
    n = 0
    for f in nc.m.functions:
        for bb in f.blocks:
            out = []
            changed = False
            for ins in bb.instructions:
                si = getattr(ins, "sync_info", None)
                waits = list(si.on_wait) if (si is not None and si.on_wait) else []
                if len(waits) > max_waits:
                    for w in waits[:-max_waits]:
                        _ctr[0] += 1
                        nop = mybir.InstNoOp(
                            name=f"I-waitsplit-{_ctr[0]}", engine=ins.engine,
                            ins=[], outs=[])
                        nop.sync_info = mybir.SyncInfo(on_wait=[w], on_update=[])
                        nc.register_instruction(nop)
                        out.append(nop)
                    si.on_wait = waits[-max_waits:]
                    changed = True
                    n += 1
                out.append(ins)
            if changed:
                bb.instructions = out
    return n


# packed f32 constant layout: ident | rcb1 | shb1 | shb2 | lng_bc | lnb_bc
CP_ID, CP_RCB1, CP_SHB1, CP_SHB2, CP_LNG, CP_LNB, CP_END = (
    0, 128, 132, 148, 152, 664, 1176)


def _build(reps=1):
    nc = bass.Bass(trn_type="TRN2")

    # ---------------- DRAM I/O ----------------
    xtp = nc.dram_tensor("xtp", [128, 4, W], F32R, kind="ExternalInput")
    xbp = nc.dram_tensor("xbp", [128, 4, W], BF16, kind="ExternalInput")
    xqp = nc.dram_tensor("xqp", [128, 2, 2, W], FP8, kind="ExternalInput")
    rcw1p = nc.dram_tensor("rcw1p", [3, 128, 4, D_IN], F32R, kind="ExternalInput")
    rcw2p = nc.dram_tensor("rcw2p", [128, 12, E], F32R, kind="ExternalInput")
    shw1p = nc.dram_tensor("shw1p", [9, 128, 4, D_HID], BF16, kind="ExternalInput")
    shw2p = nc.dram_tensor("shw2p", [128, 16, D_IN], BF16, kind="ExternalInput")
    ew1p = nc.dram_tensor("ew1p", [E, 128, 2, 2, D_HID], FP8, kind="ExternalInput")
    ew2p = nc.dram_tensor("ew2p", [E, 128, 8, 2, D_IN], FP8, kind="ExternalInput")
    ebp = nc.dram_tensor("ebp", [E, 128, 20], F32, kind="ExternalInput")
    cpack = nc.dram_tensor("cpack", [128, CP_END], F32, kind="ExternalInput")
    rcb2 = nc.dram_tensor("rcb2", [E, 1], F32, kind="ExternalInput")
    sel = nc.dram_tensor("sel", [E, E * 128], F32R, kind="ExternalInput")
    rcmaskb = nc.dram_tensor("rcmaskb", [128, 528], F32R, kind="ExternalInput")
    yout = nc.dram_tensor("yout", [4, 128, D_IN], F32, kind="ExternalOutput")

    C0 = HALO            # buffer col of first center token
    with tile.TileContext(nc) as tc:
      for rep in range(reps):
       with ExitStack() as ctx:
        R = f"r{rep}_"
        const = ctx.enter_context(tc.tile_pool(name=R + "const", bufs=1))
        acts = ctx.enter_context(tc.tile_pool(name=R + "acts", bufs=1))
        wstream = ctx.enter_context(tc.tile_pool(name=R + "wstream", bufs=2))
        estream = ctx.enter_context(tc.tile_pool(name=R + "estream", bufs=2))
        scratch = ctx.enter_context(tc.tile_pool(name=R + "scratch", bufs=2))
        hpool = ctx.enter_context(tc.tile_pool(name=R + "hpool", bufs=10))
        psum = ctx.enter_context(tc.tile_pool(name=R + "psum", bufs=7, space="PSUM"))
        pst = psum

        # ---------------- x / constants (order matters for first compute) ----
        xt_sb = acts.tile([128, 4, W], F32R, tag="xt")
        nc.sync.dma_start(xt_sb[:], xtp[:])
        xq_sb = acts.tile([128, 2, 2, W], FP8, tag="xq")
        nc.sync.dma_start(xq_sb[:], xqp[:])
        xb_sb = acts.tile([128, 4, W], BF16, tag="xb")
        nc.sync.dma_start(xb_sb[:], xbp[:])
        cp = const.tile([128, CP_END], F32)
        nc.sync.dma_start(cp[:], cpack[:])
        id_sb = cp[:, CP_ID:CP_ID + 128]
        rcb1_sb = cp[:, CP_RCB1:CP_RCB1 + 4]
        shb1_sb = cp[:, CP_SHB1:CP_SHB1 + 16]
        shb2_sb = cp[:, CP_SHB2:CP_SHB2 + 4]
        lng_bc = cp[:, CP_LNG:CP_LNG + D_IN]
        lnb_bc = cp[:, CP_LNB:CP_LNB + D_IN]
        sel_sb = const.tile([E, E * 128], F32R)
        nc.sync.dma_start(sel_sb[:], sel[:])
        rcb2_sb = const.tile([E, 1], F32)
        nc.sync.dma_start(rcb2_sb[:], rcb2[:])
        rcm_bc = const.tile([128, 528], F32R)
        nc.sync.dma_start(rcm_bc[:], rcmaskb[:])

        # =========== routing convs + top-2 gating (first: experts need gates) ==
        RC_LO, RC_W = 56, 528
        with tc.tile_pool(name=R + "routing", bufs=1) as rpool:
            rcw2_sb = rpool.tile([128, 12, E], F32R, tag="rcw2")
            nc.sync.dma_start(rcw2_sb[:], rcw2p[:])

            rc1_sb = []
            for m in range(4):
                t = rpool.tile([128, RC_W], F32R, tag=f"rc1_{m}")
                rc1_sb.append(t)
            for t0, tw in ((0, 264), (264, 264)):
                plist = [psum.tile([128, tw], F32, tag="mm", name=f"prc_{t0}_{i}")
                         for i in range(4)]
                for tap in range(3):
                    wsub = wstream.tile([128, 4, D_IN], F32R, tag="wrc", name="wrc",
                                        bufs=2)
                    nc.sync.dma_start(wsub[:], rcw1p[tap])
                    for k in range(4):
                        for m in range(4):
                            nc.tensor.matmul(
                                plist[m][:], wsub[:, k, m * 128:(m + 1) * 128],
                                xt_sb[:, k, RC_LO + t0 + tap - 1: RC_LO + t0 + tap - 1 + tw],
                                start=(tap == 0 and k == 0), stop=(tap == 2 and k == 3))
                for m in range(4):
                    nc.scalar.activation(rc1_sb[m][:, t0:t0 + tw], plist[m][:], AF.Gelu,
                                         bias=rcb1_sb[:, m:m + 1])
                    nc.vector.tensor_tensor(rc1_sb[m][:, t0:t0 + tw],
                                            rc1_sb[m][:, t0:t0 + tw],
                                            rcm_bc[:, t0:t0 + tw], op=OP.mult)

            lgp = pst.tile([E, TPC], F32, tag="aux", bufs=1, name="lgp")
            first = True
            for tap in range(3):
                for k in range(4):
                    nc.tensor.matmul(
                        lgp[:], rcw2_sb[:, tap * 4 + k, :],
                        rc1_sb[k][:, (C0 - RC_LO) + tap - 1: (C0 - RC_LO) + tap - 1 + TPC],
                        start=first, stop=(tap == 2 and k == 3))
                    first = False
            lg_sb = rpool.tile([E, TPC], F32, tag="lg")
            nc.scalar.activation(lg_sb[:], lgp[:], AF.Identity, bias=rcb2_sb[:])

            gatesT = acts.tile([E, TPC], F32R)
            for tt in range(4):
                tp = pst.tile([128, E], F32, tag="aux", bufs=1, name="tp")
                nc.tensor.transpose(tp[:], lg_sb[:, tt * 128:(tt + 1) * 128],
                                    id_sb[0:E, 0:E])
                lT = scratch.tile([128, E], F32, tag="lT")
                nc.scalar.copy(lT[:], tp[:])
                m1 = scratch.tile([128, 1], F32, tag="m1")
                nc.vector.reduce_max(m1[:], lT[:], axis=AX.X)
                mask1 = scratch.tile([128, E], F32, tag="mask1")
                nc.vector.tensor_scalar(mask1[:], lT[:], m1[:], None, op0=OP.is_equal)
                lmask = scratch.tile([128, E], F32, tag="lmask")
                nc.vector.scalar_tensor_tensor(lmask[:], mask1[:], -1e30, lT[:],
                                               op0=OP.mult, op1=OP.add)
                m2 = scratch.tile([128, 1], F32, tag="m2")
                nc.vector.reduce_max(m2[:], lmask[:], axis=AX.X)
                mask2 = scratch.tile([128, E], F32, tag="mask2")
                nc.vector.tensor_scalar(mask2[:], lmask[:], m2[:], None, op0=OP.is_equal)
                d = scratch.tile([128, 1], F32, tag="d")
                nc.vector.tensor_scalar(d[:], m2[:], m1[:], None, op0=OP.subtract)
                e_ = scratch.tile([128, 1], F32, tag="e_")
                nc.scalar.activation(e_[:], d[:], AF.Exp)
                ope = scratch.tile([128, 1], F32, tag="ope")
                nc.vector.tensor_scalar(ope[:], e_[:], 1.0, None, op0=OP.add)
                g1 = scratch.tile([128, 1], F32, tag="g1")
                nc.vector.reciprocal(g1[:], ope[:])
                g2 = scratch.tile([128, 1], F32, tag="g2")
                nc.vector.tensor_scalar(g2[:], g1[:], -1.0, 1.0, op0=OP.mult, op1=OP.add)
                t1 = scratch.tile([128, E], F32, tag="t1")
                nc.vector.tensor_scalar(t1[:], mask1[:], g1[:], None, op0=OP.mult)
                gt = scratch.tile([128, E], F32, tag="gt")
                nc.vector.scalar_tensor_tensor(gt[:], mask2[:], g2[:], t1[:],
                                               op0=OP.mult, op1=OP.add)
                gp = pst.tile([E, 128], F32, tag="aux", bufs=1, name="gp")
                nc.tensor.transpose(gp[:], gt[:], id_sb[:])
                nc.vector.tensor_scalar(gatesT[:, tt * 128:(tt + 1) * 128], gp[:],
                                        0.0, None, op0=OP.add)

        # =========== shared conv1 blocks (interleaved into the expert loop) ====
        # sh1 output: 16 bf16 tiles [128, 512]; computed as 8 dh-halves x 9 taps
        sh1_sb = []
        for m in range(16):
            t = acts.tile([128, TPC], BF16, tag=f"sh1_{m}")
            sh1_sb.append(t)
        sh1_state = {"blk": 0, "plist": None}
        N_BLK = 72            # 8 halves * 9 taps

        def emit_sh1_block():
            i = sh1_state["blk"]
            if i >= N_BLK:
                return
            sh1_state["blk"] = i + 1
            half, tap = divmod(i, 9)
            if tap == 0:
                sh1_state["plist"] = [
                    psum.tile([128, TPC], F32, tag="mm", name=f"psh_{half}_{j}")
                    for j in range(2)]
            plist = sh1_state["plist"]
            wsub = wstream.tile([128, 4, 256], BF16, tag="wsh1", name="wsh1",
                                bufs=3)
            nc.sync.dma_start(
                wsub[:], shw1p[tap, :, :, half * 256:(half + 1) * 256])
            for k in range(4):
                for mi in range(2):
                    nc.tensor.matmul(
                        plist[mi][:], wsub[:, k, mi * 128:(mi + 1) * 128],
                        xb_sb[:, k, C0 + tap - 4: C0 + tap - 4 + TPC],
                        start=(tap == 0 and k == 0), stop=(tap == 8 and k == 3))
            if tap == 8:
                for mi in range(2):
                    m = half * 2 + mi
                    nc.scalar.activation(sh1_sb[m][:], plist[mi][:], AF.Silu,
                                         bias=shb1_sb[:, m:m + 1])
                sh1_state["plist"] = None

        # =========== experts (dense fp8 DoubleRow, gated accumulate) ===========
        y_acc = [acts.tile([128, TPC], BF16, tag=f"y{mo}", name=f"y_acc_{mo}")
                 for mo in range(4)]
        for e in range(E):
            eb_sb = scratch.tile([128, 20], F32, tag="eb")
            nc.sync.dma_start(eb_sb[:], ebp[e])
            w1t = estream.tile([128, 2, 2, D_HID], FP8, tag="we1", name="we1")
            nc.sync.dma_start(w1t[:], ew1p[e])
            w2t = estream.tile([128, 8, 2, D_IN], FP8, tag="we2", name="we2")
            nc.sync.dma_start(w2t[:], ew2p[e])

            # gate row broadcast (1/4 W2 scale folded in)
            bcp = pst.tile([128, TPC], F32, tag="aux", bufs=1, name="bcp")
            nc.tensor.matmul(bcp[:], sel_sb[:, e * 128:(e + 1) * 128], gatesT[:],
                             start=True, stop=True)
            bc_sb = scratch.tile([128, TPC], F32, tag="bc")
            nc.scalar.activation(bc_sb[:], bcp[:], AF.Copy, scale=0.25)

            h_sb = [hpool.tile([128, 2, TPC], FP8, tag="h", name=f"h_{e}_{i}")
                    for i in range(8)]
            for half in range(8):      # dh-halves: chunks 2*half, 2*half+1
                plist = [psum.tile([128, TPC], F32, tag="mm",
                                   name=f"ph_{e}_{half}_{i}") for i in range(2)]
                for p in range(2):
                    for mi in range(2):
                        c = half * 256 + mi * 128
                        nc.tensor.matmul(
                            plist[mi][:], w1t[:, p, :, c:c + 128],
                            xq_sb[:, p, :, C0:C0 + TPC],
                            start=(p == 0), stop=(p == 1), perf_mode=DR)
                for mi in range(2):
                    m = half * 2 + mi
                    # elu(v) = max(v, min(exp(v), 1) - 1), v = h + b1
                    eh = scratch.tile([128, TPC], BF16, tag="he")
                    nc.scalar.activation(eh[:], plist[mi][:], AF.Exp,
                                         bias=eb_sb[:, m:m + 1])
                    em = scratch.tile([128, TPC], BF16, tag="hm")
                    nc.gpsimd.tensor_scalar(em[:], eh[:], 1.0, -1.0,
                                            op0=OP.min, op1=OP.add)
                    nc.vector.scalar_tensor_tensor(h_sb[half][:, mi, :],
                                                   plist[mi][:],
                                                   eb_sb[:, m:m + 1], em[:],
                                                   op0=OP.add, op1=OP.max)
                emit_sh1_block()

            for half in range(2):      # d_out halves for W2
                elist = [psum.tile([128, TPC], F32, tag="mm",
                                   name=f"pe_{e}_{half}_{i}") for i in range(2)]
                for q in range(8):
                    for mi in range(2):
                        mo = half * 2 + mi
                        nc.tensor.matmul(elist[mi][:],
                                         w2t[:, q, :, mo * 128:(mo + 1) * 128],
                                         h_sb[q][:], start=(q == 0), stop=(q == 7),
                                         perf_mode=DR)
                for mi in range(2):
                    mo = half * 2 + mi
                    # (eo + (4 e_b2 - colsum)) * gate/4, accumulated into y_acc
                    if e == 0:
                        nc.vector.scalar_tensor_tensor(
                            y_acc[mo][:], elist[mi][:], eb_sb[:, 16 + mo:17 + mo],
                            bc_sb[:], op0=OP.add, op1=OP.mult)
                    else:
                        yt = scratch.tile([128, TPC], BF16, tag="yt")
                        nc.vector.scalar_tensor_tensor(
                            yt[:], elist[mi][:], eb_sb[:, 16 + mo:17 + mo],
                            bc_sb[:], op0=OP.add, op1=OP.mult)
                        nc.vector.tensor_tensor(y_acc[mo][:], y_acc[mo][:], yt[:],
                                                op=OP.add)
                emit_sh1_block()

        while sh1_state["blk"] < N_BLK:
            emit_sh1_block()

        # =========== shared conv2 (k=1, bf16): sh2 kept f32 for final combine ==
        shw2_sb = const.tile([128, 16, D_IN], BF16)
        nc.sync.dma_start(shw2_sb[:], shw2p[:])
        sh2_sb = []
        for mo in range(4):
            t = acts.tile([128, TPC], F32, tag=f"sh2_{mo}")
            sh2_sb.append(t)
        for half in range(2):
            s2list = [psum.tile([128, TPC], F32, tag="mm",
                                name=f"ps2_{half}_{i}") for i in range(2)]
            for k in range(16):
                for mi in range(2):
                    mo = half * 2 + mi
                    nc.tensor.matmul(s2list[mi][:],
                                     shw2_sb[:, k, mo * 128:(mo + 1) * 128],
                                     sh1_sb[k][:], start=(k == 0), stop=(k == 15))
            for mi in range(2):
                mo = half * 2 + mi
                nc.scalar.activation(sh2_sb[mo][:], s2list[mi][:], AF.Identity,
                                     bias=shb2_sb[:, mo:mo + 1])

        # =========== z = x + y + sh2 (into sh2 tiles) ; transpose ; LN ; out ===
        z_sb = sh2_sb
        for mo in range(4):
            nc.vector.tensor_tensor(z_sb[mo][:], y_acc[mo][:], sh2_sb[mo][:],
                                    op=OP.add)
            nc.vector.tensor_tensor(z_sb[mo][:], z_sb[mo][:],
                                    xt_sb[:, mo, C0:C0 + TPC].bitcast(F32), op=OP.add)

        for tt in range(4):
            zT = scratch.tile([128, D_IN], F32, tag="zT", bufs=1)
            for mo in range(4):
                ztp = pst.tile([128, 128], F32, tag="aux", bufs=1, name="ztp")
                nc.tensor.transpose(ztp[:], z_sb[mo][:, tt * 128:(tt + 1) * 128],
                                    id_sb[:])
                nc.scalar.copy(zT[:, mo * 128:(mo + 1) * 128], ztp[:])
            srow = scratch.tile([128, 1], F32, tag="srow")
            nc.vector.reduce_sum(srow[:], zT[:], axis=AX.X)
            nmean = scratch.tile([128, 1], F32, tag="nmean")
            nc.vector.tensor_scalar(nmean[:], srow[:], -1.0 / D_IN, None, op0=OP.mult)
            zc = scratch.tile([128, D_IN], F32, tag="zc", bufs=1)
            nc.vector.tensor_scalar(zc[:], zT[:], nmean[:], None, op0=OP.add)
            sq = scratch.tile([128, D_IN], F32, tag="sq", bufs=1)
            ssq = scratch.tile([128, 1], F32, tag="ssq")
            nc.scalar.activation(sq[:], zc[:], AF.Square, accum_out=ssq[:])
            vpe = scratch.tile([128, 1], F32, tag="vpe")
            nc.vector.tensor_scalar(vpe[:], ssq[:], 1.0 / D_IN, 1e-5,
                                    op0=OP.mult, op1=OP.add)
            rinv = scratch.tile([128, 1], F32, tag="rinv")
            nc.vector.reciprocal(rinv[:], vpe[:])
            rstd = scratch.tile([128, 1], F32, tag="rstd")
            nc.scalar.activation(rstd[:], rinv[:], AF.Sqrt)
            normed = scratch.tile([128, D_IN], F32, tag="normed", bufs=1)
            nc.vector.tensor_scalar(normed[:], zc[:], rstd[:], None, op0=OP.mult)
            og = scratch.tile([128, D_IN], F32, tag="og", bufs=2)
            nc.vector.tensor_tensor(og[:], normed[:], lng_bc[:], op=OP.mult)
            nc.vector.tensor_tensor(og[:], og[:], lnb_bc[:], op=OP.add)
            nc.sync.dma_start(yout[tt], og[:])

    _split_multi_waits(nc)
    return nc


_CACHE = {}


def _get_nc(reps=1):
    key = f"nc{reps}"
    if key not in _CACHE:
        _CACHE[key] = _build(reps)
    return _CACHE[key]


LAST_RESULT = {}


def _fp8(a):
    return np.clip(np.asarray(a, np.float32), -240, 240).astype(ml_dtypes.float8_e4m3)


def _bf16(a):
    return np.asarray(a, np.float32).astype(ml_dtypes.bfloat16)


def kernel(x, rc_w1, rc_b1, rc_w2, rc_b2, sh_w1, sh_b1, sh_w2, sh_b2,
           e_w1, e_b1, e_w2, e_b2, ln_g, ln_b, **kwargs):
    x = np.asarray(x, np.float32)
    f = lambda a: np.ascontiguousarray(np.asarray(a, np.float32))
    cont = np.ascontiguousarray

    # expert weights: fp8 DoubleRow layout [E, 128, kpair, 2, out]
    ew1 = np.asarray(e_w1, np.float32)          # [E, 512, 2048]
    ew1p = _fp8(cont((4.0 * ew1).reshape(E, 2, 2, 128, D_HID)
                     .transpose(0, 3, 1, 2, 4)))
    ew2 = np.asarray(e_w2, np.float32)          # [E, 2048, 512]
    ew2p = _fp8(cont((4.0 * ew2).reshape(E, 8, 2, 128, D_IN)
                     .transpose(0, 3, 1, 2, 4)))

    # packed per-expert biases [E, 128, 20]: cols 0:16 = e_b1, 16:20 = 4*e_b2
    ebp = np.zeros((E, 128, 20), np.float32)
    ebp[:, :, :16] = np.asarray(e_b1, np.float32).reshape(E, 16, 128).transpose(0, 2, 1)
    ebp[:, :, 16:] = 4.0 * np.asarray(e_b2, np.float32).reshape(E, 4, 128).transpose(0, 2, 1)

    # packed f32 constants
    cpack = np.zeros((128, CP_END), np.float32)
    cpack[:, CP_ID:CP_ID + 128] = np.eye(128, dtype=np.float32)
    cpack[:, CP_RCB1:CP_RCB1 + 4] = np.asarray(rc_b1, np.float32).reshape(4, 128).T
    cpack[:, CP_SHB1:CP_SHB1 + 16] = np.asarray(sh_b1, np.float32).reshape(16, 128).T
    cpack[:, CP_SHB2:CP_SHB2 + 4] = np.asarray(sh_b2, np.float32).reshape(4, 128).T
    cpack[:, CP_LNG:CP_LNG + D_IN] = np.broadcast_to(
        np.asarray(ln_g, np.float32).reshape(1, D_IN), (128, D_IN))
    cpack[:, CP_LNB:CP_LNB + D_IN] = np.broadcast_to(
        np.asarray(ln_b, np.float32).reshape(1, D_IN), (128, D_IN))

    rcw1t = np.asarray(rc_w1, np.float32).transpose(2, 1, 0)   # [3, 512, 512]
    rcw1p = f(rcw1t.reshape(3, 4, 128, D_IN).transpose(0, 2, 1, 3))
    rcw2t = np.asarray(rc_w2, np.float32).transpose(2, 1, 0)   # [3, 512, 10]
    rcw2p = f(rcw2t.reshape(3, 4, 128, E).transpose(2, 0, 1, 3).reshape(128, 12, E))
    shw1t = np.asarray(sh_w1, np.float32).transpose(2, 1, 0)   # [9, 512, 2048]
    shw1p = cont(_bf16(shw1t.reshape(9, 4, 128, D_HID).transpose(0, 2, 1, 3)))
    shw2t = np.asarray(sh_w2, np.float32)[:, :, 0].T           # [2048, 512]
    shw2p = cont(_bf16(shw2t.reshape(16, 128, D_IN).transpose(1, 0, 2)))

    shared = {
        "rcw1p": rcw1p,
        "rcw2p": rcw2p,
        "shw1p": shw1p,
        "shw2p": shw2p,
        "ew1p": ew1p,
        "ew2p": ew2p,
        "ebp": ebp,
        "cpack": cpack,
        "rcb2": f(np.asarray(rc_b2, np.float32).reshape(E, 1)),
        "sel": np.repeat(np.eye(E, dtype=np.float32), 128, axis=1),
    }

    in_maps = []
    for c in range(NCORES):
        b, j = divmod(c, T // TPC)
        lo_tok = j * TPC - HALO
        hi_tok = j * TPC + TPC + HALO
        xh = np.zeros((W, D_IN), np.float32)
        lo = max(0, lo_tok)
        hi = min(T, hi_tok)
        xh[lo - lo_tok: hi - lo_tok] = x[b, lo:hi]
        im = dict(shared)
        xhT = np.ascontiguousarray(xh.T)            # [512, W]
        x4 = cont(xhT.reshape(4, 128, W).transpose(1, 0, 2))   # [128, 4, W]
        im["xtp"] = x4
        im["xbp"] = cont(_bf16(x4))
        im["xqp"] = _fp8(cont((xhT / 4.0).reshape(2, 2, 128, W)
                              .transpose(2, 0, 1, 3)))
        rcm = np.zeros((1, 528), np.float32)
        for cidx in range(528):
            tok = j * TPC - 8 + cidx
            if 0 <= tok < T:
                rcm[0, cidx] = 1.0
        im["rcmaskb"] = np.ascontiguousarray(np.broadcast_to(rcm, (128, 528)))
        in_maps.append(im)

    nc = _get_nc()
    res = run_bass_kernel_spmd(nc, in_maps, core_ids=list(range(NCORES)),
                               **kwargs)
    LAST_RESULT["res"] = res

    out = np.empty((B, T, D_IN), np.float32)
    for c in range(NCORES):
        b, j = divmod(c, T // TPC)
        out[b, j * TPC:(j + 1) * TPC] = res.results[c]["yout"].reshape(TPC, D_IN)
    return out
